# revision 9
# baseline (speedup 1.0000x reference)
"""Multi-head causal attention on 8 Trainium2 cores (Bass/Tile).

Problem: B=4, S=2048, D=2048, H=16 heads of dim 128, causal, fp32.
  q,k,v = x@Wq, x@Wk, x@Wv  (split heads); scores=q@k^T (causal mask, /sqrt(128));
  out = softmax @ v (merged) @ Wo + bo.

Sharding (8 cores): core c -> (batch b=c//2, head-half hg=c%2).
Each core computes its batch's attention for 8 of the 16 heads plus the
partial output projection for those heads' rows of Wo. Host sums the two
partials per batch and adds the bias (the tensor-parallel all-reduce
degenerates to the unshard step since outputs are partial sums).

Per-core kernel (all matmuls in float32r = full-rate PE):
  - 4 head-groups of 2 heads. Per group, per sq-chunk j (4 x 512):
      QT/KT ([hd,seq], via lhsT=W, rhs=x^T) and V ([seq,hd], via lhsT=x^T,
      rhs=W) projections accumulated over 16 k-chunks of d.
      Attention: S^T tiles [sk 128, sq 512] = K-chunk @ Q^T; exp on ScalarE
      (scale=1/sqrt(128)); causal handled by skipping fully-masked tiles,
      narrowing straddling tiles to [128r:512], and a [128,128] triangular
      mask multiply on the diagonal block; denominator via ones-vector
      matmul accumulated in PSUM; ctx^T = V^T @ P^T accumulated in PSUM;
      normalization = reciprocal + partition_broadcast + multiply (writes
      ctx^T straight to SBUF).
  - Per-group output projection out_g = ctx_g @ Wo_g -> its own DRAM
    output; host sums the 4 partials (avoids a big resident ctx buffer).
"""

import numpy as np

import concourse.bass as bass
import concourse.mybir as mybir
import concourse.tile as tile
from concourse import bacc
from concourse.bass_utils import run_bass_kernel_spmd
from concourse.masks import make_upper_triangular

F32 = mybir.dt.float32
F32R = mybir.dt.float32r
EXP = mybir.ActivationFunctionType.Exp
MULT = mybir.AluOpType.mult

B, S, D = 4, 2048, 2048
HD = 128          # head dim
NH = 8            # heads per core
G = 2             # heads per group
NG = NH // G      # 4 groups
SQ = 512          # sq chunk (matmul moving dim)
NSQ = S // SQ     # 4
NK = D // 128     # 16 contraction chunks
DH = D // 2       # 1024 = per-core slice of d_out for q/k/v
SCALE = 1.0 / float(np.sqrt(HD))


DEBUG_DUMPS = False


def _build():
    nc = bacc.Bacc("TRN2", target_bir_lowering=False, debug=False, num_devices=8)

    xt = nc.dram_tensor("xt", [D, S], F32R, kind="ExternalInput")      # x^T (d, seq)
    wq = nc.dram_tensor("wq", [D, DH], F32R, kind="ExternalInput")
    wk = nc.dram_tensor("wk", [D, DH], F32R, kind="ExternalInput")
    wv = nc.dram_tensor("wv", [D, DH], F32R, kind="ExternalInput")
    wo = nc.dram_tensor("wo", [DH, D], F32R, kind="ExternalInput")
    outs = [
        nc.dram_tensor(f"out{g}", [S, D], F32, kind="ExternalOutput")
        for g in range(NG)
    ]
    dbg = {}
    if DEBUG_DUMPS:
        dbg["qt"] = nc.dram_tensor("dbg_qt", [128, SQ], F32R, kind="ExternalOutput")
        dbg["kt"] = nc.dram_tensor("dbg_kt", [128, S], F32R, kind="ExternalOutput")
        dbg["v2"] = nc.dram_tensor(
            "dbg_v2", [128, NK, G * HD], F32R, kind="ExternalOutput"
        )
        dbg["ctx"] = nc.dram_tensor(
            "dbg_ctx", [128, G, S], F32R, kind="ExternalOutput"
        )
        dbg["pt"] = nc.dram_tensor("dbg_pt", [128, SQ], F32R, kind="ExternalOutput")
        dbg["rsb"] = nc.dram_tensor("dbg_rsb", [1, SQ], F32, kind="ExternalOutput")

    with tile.TileContext(nc) as tc:
        with (
            tc.tile_pool(name="const", bufs=1) as constp,
            tc.tile_pool(name="wqkv", bufs=1) as wpool,
            tc.tile_pool(name="ktv", bufs=1) as ktvp,
            tc.tile_pool(name="qt", bufs=4) as qtp,
            tc.tile_pool(name="xt", bufs=16) as xtp,
            tc.tile_pool(name="pt", bufs=3) as ptp,
            tc.tile_pool(name="ctxT", bufs=2) as ctxp,
            tc.tile_pool(name="wop", bufs=2) as wop,
            tc.tile_pool(name="osb", bufs=3) as osbp,
            tc.tile_pool(name="small", bufs=2) as smallp,
            tc.tile_pool(name="ps_proj", bufs=2, space="PSUM") as ps_proj,
            tc.tile_pool(name="ps_st", bufs=2, space="PSUM") as ps_st,
            tc.tile_pool(name="ps_ctx", bufs=2, space="PSUM") as ps_ctx,
            tc.tile_pool(name="ps_d", bufs=1, space="PSUM") as ps_d,
            tc.tile_pool(name="ps_out", bufs=1, space="PSUM") as ps_out,
        ):
            # constants
            tri32 = constp.tile([128, 128], F32, name="tri32")
            make_upper_triangular(nc, tri32[:], val=1.0, diag=True)
            ones32 = constp.tile([128, 1], F32, name="ones32")
            nc.vector.memset(ones32[:], 1.0)
            ones_r = constp.tile([128, 1], F32R, name="ones_r")
            nc.vector.tensor_copy(ones_r[:], ones32[:])

            for g in range(NG):
                # per-group weight slices [128, NK, 256], d on partitions
                wq_t = wpool.tile([128, NK, G * HD], F32R, tag="wq", name=f"wq{g}")
                wk_t = wpool.tile([128, NK, G * HD], F32R, tag="wk", name=f"wk{g}")
                wv_t = wpool.tile([128, NK, G * HD], F32R, tag="wv", name=f"wv{g}")
                for w_sb, w_dr in ((wq_t, wq), (wk_t, wk), (wv_t, wv)):
                    src = w_dr.ap()[:, g * G * HD:(g + 1) * G * HD]
                    nc.sync.dma_start(
                        w_sb[:], src.rearrange("(o p) n -> p o n", p=128)
                    )

                kt = [
                    ktvp.tile([128, S], F32R, tag=f"kt{t}", name=f"kt{g}_{t}")
                    for t in range(G)
                ]
                v2 = ktvp.tile([128, NK, G * HD], F32R, tag="v2", name=f"v2{g}")
                ctx_g = ctxp.tile([128, G, S], F32R, tag="ctx", name=f"ctx{g}")

                for j in range(NSQ):
                    xts = []
                    for k in range(NK):
                        t_ = xtp.tile([128, SQ], F32R, tag="xt", name=f"x{g}{j}{k}")
                        nc.sync.dma_start(
                            t_[:],
                            xt.ap()[k * 128:(k + 1) * 128, j * SQ:(j + 1) * SQ],
                        )
                        xts.append(t_)

                    # ---- pass Q: QT[t] [hd=128, sq 512]
                    pq = [
                        ps_proj.tile([128, SQ], F32, tag="proj", name=f"pq{t}")
                        for t in range(G)
                    ]
                    for k in range(NK):
                        for t in range(G):
                            nc.tensor.matmul(
                                pq[t][:],
                                wq_t[:, k, t * HD:(t + 1) * HD],
                                xts[k][:],
                                start=(k == 0),
                                stop=(k == NK - 1),
                            )
                    qt = []
                    for t in range(G):
                        q_ = qtp.tile([128, SQ], F32R, tag="qt", name=f"qt{t}")
                        nc.any.tensor_copy(q_[:], pq[t][:])
                        qt.append(q_)
                    if DEBUG_DUMPS and g == 0 and j == 0:
                        nc.sync.dma_start(dbg["qt"].ap(), qt[0][:])

                    # ---- pass K: KT[t][:, j*SQ:+SQ]
                    pk = [
                        ps_proj.tile([128, SQ], F32, tag="proj", name=f"pk{t}")
                        for t in range(G)
                    ]
                    for k in range(NK):
                        for t in range(G):
                            nc.tensor.matmul(
                                pk[t][:],
                                wk_t[:, k, t * HD:(t + 1) * HD],
                                xts[k][:],
                                start=(k == 0),
                                stop=(k == NK - 1),
                            )
                    for t in range(G):
                        nc.any.tensor_copy(kt[t][:, j * SQ:(j + 1) * SQ], pk[t][:])

                    # ---- pass V: V[sq 128, 2*HD] for 4 sq-subchunks.
                    # One accumulation group per PSUM bank: start=True clears
                    # the whole bank, so groups must not share one.
                    for half in range(2):
                        pv = [
                            ps_proj.tile([128, 256], F32, tag="proj", name=f"pv{h}")
                            for h in range(2)
                        ]
                        for k in range(NK):
                            for h in range(2):
                                s_ = 2 * half + h
                                nc.tensor.matmul(
                                    pv[h][:],
                                    xts[k][:, s_ * 128:(s_ + 1) * 128],
                                    wv_t[:, k, :],
                                    start=(k == 0),
                                    stop=(k == NK - 1),
                                )
                        for h in range(2):
                            s_ = 2 * half + h
                            nc.any.tensor_copy(v2[:, 4 * j + s_, :], pv[h][:])

                    # ---- attention for both heads at this j
                    n_sk = 4 * (j + 1)
                    for t in range(G):
                        dps = ps_d.tile([1, SQ], F32, tag="d", name="dps")
                        cps = ps_ctx.tile([128, SQ], F32, tag="ctx", name="cps")
                        for i in range(n_sk):
                            r = i - 4 * j  # >=0: straddles the causal diagonal
                            lo = 128 * r if r > 0 else 0
                            st = ps_st.tile([128, SQ], F32, tag="st", name="st")
                            nc.tensor.matmul(
                                st[:, lo:],
                                kt[t][:, i * 128:(i + 1) * 128],
                                qt[t][:, lo:],
                                start=True,
                                stop=True,
                            )
                            pt = ptp.tile([128, SQ], F32R, tag="pt", name="pt")
                            nc.scalar.activation(
                                pt[:, lo:], st[:, lo:], EXP, scale=SCALE
                            )
                            if r >= 0:
                                nc.vector.tensor_tensor(
                                    pt[:, lo:lo + 128],
                                    pt[:, lo:lo + 128],
                                    tri32[:],
                                    MULT,
                                )
                            if DEBUG_DUMPS and g == 0 and t == 0 and j == 0 and i == 0:
                                nc.sync.dma_start(dbg["pt"].ap(), pt[:])
                            nc.tensor.matmul(
                                dps[0:1, lo:],
                                ones_r[:],
                                pt[:, lo:],
                                start=(i == 0),
                                stop=(i == n_sk - 1),
                            )
                            nc.tensor.matmul(
                                cps[:, lo:],
                                v2[:, i, t * HD:(t + 1) * HD],
                                pt[:, lo:],
                                start=(i == 0),
                                stop=(i == n_sk - 1),
                            )
                        # normalize: ctx_g[:, t, j*SQ:+SQ] = cps / d
                        rsb = smallp.tile([1, SQ], F32, tag="rsb", name="rsb")
                        nc.vector.reciprocal(rsb[:], dps[:])
                        if DEBUG_DUMPS and g == 0 and t == 0 and j == 0:
                            nc.sync.dma_start(dbg["rsb"].ap(), rsb[:])
                        rrep = smallp.tile([128, SQ], F32, tag="rrep", name="rrep")
                        nc.gpsimd.partition_broadcast(rrep[:], rsb[:])
                        nc.vector.tensor_tensor(
                            ctx_g[:, t, j * SQ:(j + 1) * SQ], cps[:], rrep[:], MULT
                        )

                if DEBUG_DUMPS and g == 0:
                    nc.sync.dma_start(dbg["kt"].ap(), kt[0][:])
                    nc.sync.dma_start(dbg["v2"].ap(), v2[:])
                    nc.sync.dma_start(dbg["ctx"].ap(), ctx_g[:])

                # ---- per-group output projection: out_g = ctx_g @ Wo_g
                for m in range(4):
                    wo_m = wop.tile([128, G, SQ], F32R, tag="wo", name=f"wo{m}")
                    for t in range(G):
                        row0 = g * G * HD + t * HD
                        nc.sync.dma_start(
                            wo_m[:, t, :],
                            wo.ap()[row0:row0 + 128, m * SQ:(m + 1) * SQ],
                        )
                    for s_ in range(S // 128):
                        ops = ps_out.tile([128, SQ], F32, tag="outp", name="ops")
                        for t in range(G):
                            nc.tensor.matmul(
                                ops[:],
                                ctx_g[:, t, s_ * 128:(s_ + 1) * 128],
                                wo_m[:, t, :],
                                start=(t == 0),
                                stop=(t == G - 1),
                            )
                        osb = osbp.tile([128, SQ], F32, tag="osb", name="osb")
                        nc.any.tensor_copy(osb[:], ops[:])
                        nc.sync.dma_start(
                            outs[g].ap()[s_ * 128:(s_ + 1) * 128, m * SQ:(m + 1) * SQ],
                            osb[:],
                        )

    nc.compile()
    return nc


_NC = None


def _get_nc():
    global _NC
    if _NC is None:
        _NC = _build()
    return _NC


def kernel(x, W_q, W_k, W_v, W_o, b_o):
    x = np.asarray(x, dtype=np.float32)
    W_q = np.asarray(W_q, dtype=np.float32)
    W_k = np.asarray(W_k, dtype=np.float32)
    W_v = np.asarray(W_v, dtype=np.float32)
    W_o = np.asarray(W_o, dtype=np.float32)
    b_o = np.asarray(b_o, dtype=np.float32)

    nc = _get_nc()
    in_maps = []
    for c in range(8):
        b, hg = divmod(c, 2)
        lo = hg * DH
        in_maps.append(
            {
                "xt": np.ascontiguousarray(x[b].T),
                "wq": np.ascontiguousarray(W_q[:, lo:lo + DH]),
                "wk": np.ascontiguousarray(W_k[:, lo:lo + DH]),
                "wv": np.ascontiguousarray(W_v[:, lo:lo + DH]),
                "wo": np.ascontiguousarray(W_o[lo:lo + DH, :]),
            }
        )

    res = run_bass_kernel_spmd(nc, in_maps, core_ids=list(range(8)))

    out = np.zeros((B, S, D), dtype=np.float32)
    for c in range(8):
        b = c // 2
        r = res.results[c]
        for g in range(NG):
            out[b] += r[f"out{g}"]
    out += b_o[None, None, :]
    return out


# revision 12
# speedup vs baseline: 1.0708x; 1.0708x over previous
"""Multi-head causal attention on 8 Trainium2 cores (Bass/Tile).

Problem: B=4, S=2048, D=2048, H=16 heads of dim 128, causal, fp32.
  q,k,v = x@Wq, x@Wk, x@Wv  (split heads); scores=q@k^T (causal mask, /sqrt(128));
  out = softmax @ v (merged) @ Wo + bo.

Sharding (8 cores): core c -> (batch b=c//2, head-half hg=c%2).
Each core computes its batch's attention for 8 of the 16 heads plus the
partial output projection for those heads' rows of Wo. Host sums the two
partials per batch and adds the bias (the tensor-parallel all-reduce
degenerates to the unshard step since outputs are partial sums).

Per-core kernel (all matmuls in float32r = full-rate PE):
  - 4 head-groups of 2 heads. Per group, per sq-chunk j (4 x 512):
      QT/KT ([hd,seq], via lhsT=W, rhs=x^T) and V ([seq,hd], via lhsT=x^T,
      rhs=W) projections accumulated over 16 k-chunks of d.
      Attention: S^T tiles [sk 128, sq 512] = K-chunk @ Q^T; exp on ScalarE
      (scale=1/sqrt(128)); causal handled by skipping fully-masked tiles,
      narrowing straddling tiles to [128r:512], and a [128,128] triangular
      mask multiply on the diagonal block; denominator via ones-vector
      matmul accumulated in PSUM; ctx^T = V^T @ P^T accumulated in PSUM;
      normalization = reciprocal + partition_broadcast + multiply (writes
      ctx^T straight to SBUF).
  - Per-group output projection out_g = ctx_g @ Wo_g -> its own DRAM
    output; host sums the 4 partials (avoids a big resident ctx buffer).
"""

import numpy as np

import concourse.bass as bass
import concourse.mybir as mybir
import concourse.tile as tile
from concourse import bacc
from concourse.bass_utils import run_bass_kernel_spmd
from concourse.masks import make_upper_triangular

F32 = mybir.dt.float32
F32R = mybir.dt.float32r
EXP = mybir.ActivationFunctionType.Exp
MULT = mybir.AluOpType.mult

B, S, D = 4, 2048, 2048
HD = 128          # head dim
NH = 8            # heads per core
G = 2             # heads per group
NG = NH // G      # 4 groups
SQ = 512          # sq chunk (matmul moving dim)
NSQ = S // SQ     # 4
NK = D // 128     # 16 contraction chunks
DH = D // 2       # 1024 = per-core slice of d_out for q/k/v
SCALE = 1.0 / float(np.sqrt(HD))


DEBUG_DUMPS = False


def _build():
    nc = bacc.Bacc("TRN2", target_bir_lowering=False, debug=False, num_devices=8)

    xt = nc.dram_tensor("xt", [D, S], F32R, kind="ExternalInput")      # x^T (d, seq)
    wq = nc.dram_tensor("wq", [D, DH], F32R, kind="ExternalInput")
    wk = nc.dram_tensor("wk", [D, DH], F32R, kind="ExternalInput")
    wv = nc.dram_tensor("wv", [D, DH], F32R, kind="ExternalInput")
    wo = nc.dram_tensor("wo", [DH, D], F32R, kind="ExternalInput")
    outs = [
        nc.dram_tensor(f"out{g}", [S, D], F32, kind="ExternalOutput")
        for g in range(NG)
    ]
    dbg = {}
    if DEBUG_DUMPS:
        dbg["qt"] = nc.dram_tensor("dbg_qt", [128, SQ], F32R, kind="ExternalOutput")
        dbg["kt"] = nc.dram_tensor("dbg_kt", [128, S], F32R, kind="ExternalOutput")
        dbg["v2"] = nc.dram_tensor(
            "dbg_v2", [128, NK, G * HD], F32R, kind="ExternalOutput"
        )
        dbg["ctx"] = nc.dram_tensor(
            "dbg_ctx", [128, G, S], F32R, kind="ExternalOutput"
        )
        dbg["pt"] = nc.dram_tensor("dbg_pt", [128, SQ], F32R, kind="ExternalOutput")
        dbg["rsb"] = nc.dram_tensor("dbg_rsb", [1, SQ], F32, kind="ExternalOutput")

    with tile.TileContext(nc) as tc:
        with (
            tc.tile_pool(name="const", bufs=1) as constp,
            tc.tile_pool(name="wqkv", bufs=1) as wpool,
            tc.tile_pool(name="ktv", bufs=1) as ktvp,
            tc.tile_pool(name="qt", bufs=4) as qtp,
            tc.tile_pool(name="xt", bufs=16) as xtp,
            tc.tile_pool(name="pt", bufs=3) as ptp,
            tc.tile_pool(name="ctxT", bufs=2) as ctxp,
            tc.tile_pool(name="wop", bufs=2) as wop,
            tc.tile_pool(name="osb", bufs=3) as osbp,
            tc.tile_pool(name="small", bufs=2) as smallp,
            tc.tile_pool(name="ps_proj", bufs=2, space="PSUM") as ps_proj,
            tc.tile_pool(name="ps_st", bufs=2, space="PSUM") as ps_st,
            tc.tile_pool(name="ps_ctx", bufs=2, space="PSUM") as ps_ctx,
            tc.tile_pool(name="ps_d", bufs=1, space="PSUM") as ps_d,
            tc.tile_pool(name="ps_out", bufs=1, space="PSUM") as ps_out,
        ):
            # constants
            tri32 = constp.tile([128, 128], F32, name="tri32")
            make_upper_triangular(nc, tri32[:], val=1.0, diag=True)
            ones32 = constp.tile([128, 1], F32, name="ones32")
            nc.vector.memset(ones32[:], 1.0)
            ones_r = constp.tile([128, 1], F32R, name="ones_r")
            nc.vector.tensor_copy(ones_r[:], ones32[:])

            for g in range(NG):
                # per-group weight slices [128, NK, 256], d on partitions
                wq_t = wpool.tile([128, NK, G * HD], F32R, tag="wq", name=f"wq{g}")
                wk_t = wpool.tile([128, NK, G * HD], F32R, tag="wk", name=f"wk{g}")
                wv_t = wpool.tile([128, NK, G * HD], F32R, tag="wv", name=f"wv{g}")
                for w_sb, w_dr in ((wq_t, wq), (wk_t, wk), (wv_t, wv)):
                    src = w_dr.ap()[:, g * G * HD:(g + 1) * G * HD]
                    nc.sync.dma_start(
                        w_sb[:], src.rearrange("(o p) n -> p o n", p=128)
                    )

                kt = [
                    ktvp.tile([128, S], F32R, tag=f"kt{t}", name=f"kt{g}_{t}")
                    for t in range(G)
                ]
                v2 = ktvp.tile([128, NK, G * HD], F32R, tag="v2", name=f"v2{g}")
                ctx_g = ctxp.tile([128, G, S], F32R, tag="ctx", name=f"ctx{g}")

                for j in range(NSQ):
                    xts = []
                    for k in range(NK):
                        t_ = xtp.tile([128, SQ], F32R, tag="xt", name=f"x{g}{j}{k}")
                        nc.sync.dma_start(
                            t_[:],
                            xt.ap()[k * 128:(k + 1) * 128, j * SQ:(j + 1) * SQ],
                        )
                        xts.append(t_)

                    # ---- pass Q: QT[t] [hd=128, sq 512]
                    pq = [
                        ps_proj.tile([128, SQ], F32, tag="proj", name=f"pq{t}")
                        for t in range(G)
                    ]
                    for k in range(NK):
                        for t in range(G):
                            nc.tensor.matmul(
                                pq[t][:],
                                wq_t[:, k, t * HD:(t + 1) * HD],
                                xts[k][:],
                                start=(k == 0),
                                stop=(k == NK - 1),
                            )
                    qt = []
                    for t in range(G):
                        q_ = qtp.tile([128, SQ], F32R, tag="qt", name=f"qt{t}")
                        nc.vector.tensor_copy(q_[:], pq[t][:])
                        qt.append(q_)
                    if DEBUG_DUMPS and g == 0 and j == 0:
                        nc.sync.dma_start(dbg["qt"].ap(), qt[0][:])

                    # ---- pass K: KT[t][:, j*SQ:+SQ]
                    pk = [
                        ps_proj.tile([128, SQ], F32, tag="proj", name=f"pk{t}")
                        for t in range(G)
                    ]
                    for k in range(NK):
                        for t in range(G):
                            nc.tensor.matmul(
                                pk[t][:],
                                wk_t[:, k, t * HD:(t + 1) * HD],
                                xts[k][:],
                                start=(k == 0),
                                stop=(k == NK - 1),
                            )
                    for t in range(G):
                        nc.vector.tensor_copy(kt[t][:, j * SQ:(j + 1) * SQ], pk[t][:])

                    # ---- pass V: V[sq 128, 2*HD] for 4 sq-subchunks.
                    # One accumulation group per PSUM bank: start=True clears
                    # the whole bank, so groups must not share one.
                    for half in range(2):
                        pv = [
                            ps_proj.tile([128, 256], F32, tag="proj", name=f"pv{h}")
                            for h in range(2)
                        ]
                        for k in range(NK):
                            for h in range(2):
                                s_ = 2 * half + h
                                nc.tensor.matmul(
                                    pv[h][:],
                                    xts[k][:, s_ * 128:(s_ + 1) * 128],
                                    wv_t[:, k, :],
                                    start=(k == 0),
                                    stop=(k == NK - 1),
                                )
                        for h in range(2):
                            s_ = 2 * half + h
                            nc.vector.tensor_copy(v2[:, 4 * j + s_, :], pv[h][:])

                    # ---- attention for both heads at this j
                    n_sk = 4 * (j + 1)
                    for t in range(G):
                        dps = ps_d.tile([1, SQ], F32, tag="d", name="dps")
                        cps = ps_ctx.tile([128, SQ], F32, tag="ctx", name="cps")
                        for i in range(n_sk):
                            r = i - 4 * j  # >=0: straddles the causal diagonal
                            lo = 128 * r if r > 0 else 0
                            st = ps_st.tile([128, SQ], F32, tag="st", name="st")
                            nc.tensor.matmul(
                                st[:, lo:],
                                kt[t][:, i * 128:(i + 1) * 128],
                                qt[t][:, lo:],
                                start=True,
                                stop=True,
                            )
                            pt = ptp.tile([128, SQ], F32R, tag="pt", name="pt")
                            nc.scalar.activation(
                                pt[:, lo:], st[:, lo:], EXP, scale=SCALE
                            )
                            if r >= 0:
                                nc.vector.tensor_tensor(
                                    pt[:, lo:lo + 128],
                                    pt[:, lo:lo + 128],
                                    tri32[:],
                                    MULT,
                                )
                            if DEBUG_DUMPS and g == 0 and t == 0 and j == 0 and i == 0:
                                nc.sync.dma_start(dbg["pt"].ap(), pt[:])
                            nc.tensor.matmul(
                                cps[:, lo:],
                                v2[:, i, t * HD:(t + 1) * HD],
                                pt[:, lo:],
                                start=(i == 0),
                                stop=(i == n_sk - 1),
                            )
                            nc.tensor.matmul(
                                dps[0:1, lo:],
                                ones_r[:],
                                pt[:, lo:],
                                start=(i == 0),
                                stop=(i == n_sk - 1),
                            )
                        # normalize: ctx_g[:, t, j*SQ:+SQ] = cps / d
                        rsb = smallp.tile([1, SQ], F32, tag="rsb", name="rsb")
                        nc.vector.reciprocal_approx_fast(rsb[:], dps[:])
                        if DEBUG_DUMPS and g == 0 and t == 0 and j == 0:
                            nc.sync.dma_start(dbg["rsb"].ap(), rsb[:])
                        rrep = smallp.tile([128, SQ], F32, tag="rrep", name="rrep")
                        nc.gpsimd.partition_broadcast(rrep[:], rsb[:])
                        nc.vector.tensor_tensor(
                            ctx_g[:, t, j * SQ:(j + 1) * SQ], cps[:], rrep[:], MULT
                        )

                if DEBUG_DUMPS and g == 0:
                    nc.sync.dma_start(dbg["kt"].ap(), kt[0][:])
                    nc.sync.dma_start(dbg["v2"].ap(), v2[:])
                    nc.sync.dma_start(dbg["ctx"].ap(), ctx_g[:])

                # ---- per-group output projection: out_g = ctx_g @ Wo_g
                for m in range(4):
                    wo_m = wop.tile([128, G, SQ], F32R, tag="wo", name=f"wo{m}")
                    for t in range(G):
                        row0 = g * G * HD + t * HD
                        nc.sync.dma_start(
                            wo_m[:, t, :],
                            wo.ap()[row0:row0 + 128, m * SQ:(m + 1) * SQ],
                        )
                    for s_ in range(S // 128):
                        ops = ps_out.tile([128, SQ], F32, tag="outp", name="ops")
                        for t in range(G):
                            nc.tensor.matmul(
                                ops[:],
                                ctx_g[:, t, s_ * 128:(s_ + 1) * 128],
                                wo_m[:, t, :],
                                start=(t == 0),
                                stop=(t == G - 1),
                            )
                        osb = osbp.tile([128, SQ], F32, tag="osb", name="osb")
                        nc.vector.tensor_copy(osb[:], ops[:])
                        nc.sync.dma_start(
                            outs[g].ap()[s_ * 128:(s_ + 1) * 128, m * SQ:(m + 1) * SQ],
                            osb[:],
                        )

    nc.compile()
    return nc


_NC = None


def _get_nc():
    global _NC
    if _NC is None:
        _NC = _build()
    return _NC


def kernel(x, W_q, W_k, W_v, W_o, b_o):
    x = np.asarray(x, dtype=np.float32)
    W_q = np.asarray(W_q, dtype=np.float32)
    W_k = np.asarray(W_k, dtype=np.float32)
    W_v = np.asarray(W_v, dtype=np.float32)
    W_o = np.asarray(W_o, dtype=np.float32)
    b_o = np.asarray(b_o, dtype=np.float32)

    nc = _get_nc()
    in_maps = []
    for c in range(8):
        b, hg = divmod(c, 2)
        lo = hg * DH
        in_maps.append(
            {
                "xt": np.ascontiguousarray(x[b].T),
                "wq": np.ascontiguousarray(W_q[:, lo:lo + DH]),
                "wk": np.ascontiguousarray(W_k[:, lo:lo + DH]),
                "wv": np.ascontiguousarray(W_v[:, lo:lo + DH]),
                "wo": np.ascontiguousarray(W_o[lo:lo + DH, :]),
            }
        )

    res = run_bass_kernel_spmd(nc, in_maps, core_ids=list(range(8)))

    out = np.zeros((B, S, D), dtype=np.float32)
    for c in range(8):
        b = c // 2
        r = res.results[c]
        for g in range(NG):
            out[b] += r[f"out{g}"]
    out += b_o[None, None, :]
    return out


# revision 16
# speedup vs baseline: 1.0712x; 1.0003x over previous
"""Multi-head causal attention on 8 Trainium2 cores (Bass/Tile).

Problem: B=4, S=2048, D=2048, H=16 heads of dim 128, causal, fp32.
  q,k,v = x@Wq, x@Wk, x@Wv  (split heads); scores=q@k^T (causal mask, /sqrt(128));
  out = softmax @ v (merged) @ Wo + bo.

Sharding (8 cores): core c -> (batch b=c//2, head-half hg=c%2).
Each core computes its batch's attention for 8 of the 16 heads plus the
partial output projection for those heads' rows of Wo. Host sums the two
partials per batch and adds the bias (the tensor-parallel all-reduce
degenerates to the unshard step since outputs are partial sums).

Per-core kernel (all matmuls in float32r = full-rate PE):
  - 4 head-groups of 2 heads. Per group, per sq-chunk j (4 x 512):
      QT/KT ([hd,seq], via lhsT=W, rhs=x^T) and V ([seq,hd], via lhsT=x^T,
      rhs=W) projections accumulated over 16 k-chunks of d.
      Attention: S^T tiles [sk 128, sq 512] = K-chunk @ Q^T; exp on ScalarE
      (scale=1/sqrt(128)); causal handled by skipping fully-masked tiles,
      narrowing straddling tiles to [128r:512], and a [128,128] triangular
      mask multiply on the diagonal block; denominator via ones-vector
      matmul accumulated in PSUM; ctx^T = V^T @ P^T accumulated in PSUM;
      normalization = reciprocal + partition_broadcast + multiply (writes
      ctx^T straight to SBUF).
  - Per-group output projection out_g = ctx_g @ Wo_g -> its own DRAM
    output; host sums the 4 partials (avoids a big resident ctx buffer).
"""

import numpy as np

import concourse.bass as bass
import concourse.mybir as mybir
import concourse.tile as tile
from concourse import bacc
from concourse.bass_utils import run_bass_kernel_spmd
from concourse.masks import make_upper_triangular

F32 = mybir.dt.float32
F32R = mybir.dt.float32r
EXP = mybir.ActivationFunctionType.Exp
MULT = mybir.AluOpType.mult

B, S, D = 4, 2048, 2048
HD = 128          # head dim
NH = 8            # heads per core
G = 2             # heads per group
NG = NH // G      # 4 groups
SQ = 512          # sq chunk (matmul moving dim)
NSQ = S // SQ     # 4
NK = D // 128     # 16 contraction chunks
DH = D // 2       # 1024 = per-core slice of d_out for q/k/v
SCALE = 1.0 / float(np.sqrt(HD))


DEBUG_DUMPS = False


def _build():
    nc = bacc.Bacc("TRN2", target_bir_lowering=False, debug=False, num_devices=8)

    xt = nc.dram_tensor("xt", [D, S], F32R, kind="ExternalInput")      # x^T (d, seq)
    wq = nc.dram_tensor("wq", [D, DH], F32R, kind="ExternalInput")
    wk = nc.dram_tensor("wk", [D, DH], F32R, kind="ExternalInput")
    wv = nc.dram_tensor("wv", [D, DH], F32R, kind="ExternalInput")
    wo = nc.dram_tensor("wo", [DH, D], F32R, kind="ExternalInput")
    outs = [
        nc.dram_tensor(f"out{g}", [S, D], F32, kind="ExternalOutput")
        for g in range(NG)
    ]
    dbg = {}
    if DEBUG_DUMPS:
        dbg["qt"] = nc.dram_tensor("dbg_qt", [128, SQ], F32R, kind="ExternalOutput")
        dbg["kt"] = nc.dram_tensor("dbg_kt", [128, S], F32R, kind="ExternalOutput")
        dbg["v2"] = nc.dram_tensor(
            "dbg_v2", [128, NK, G * HD], F32R, kind="ExternalOutput"
        )
        dbg["ctx"] = nc.dram_tensor(
            "dbg_ctx", [128, G, S], F32R, kind="ExternalOutput"
        )
        dbg["pt"] = nc.dram_tensor("dbg_pt", [128, SQ], F32R, kind="ExternalOutput")
        dbg["rsb"] = nc.dram_tensor("dbg_rsb", [1, SQ], F32, kind="ExternalOutput")

    with tile.TileContext(nc) as tc:
        with (
            tc.tile_pool(name="const", bufs=1) as constp,
            tc.tile_pool(name="wqkv", bufs=1) as wpool,
            tc.tile_pool(name="ktv", bufs=1) as ktvp,
            tc.tile_pool(name="qt", bufs=4) as qtp,
            tc.tile_pool(name="xt", bufs=16) as xtp,
            tc.tile_pool(name="pt", bufs=3) as ptp,
            tc.tile_pool(name="ctxT", bufs=2) as ctxp,
            tc.tile_pool(name="wop", bufs=2) as wop,
            tc.tile_pool(name="osb", bufs=3) as osbp,
            tc.tile_pool(name="small", bufs=2) as smallp,
            tc.tile_pool(name="ps_proj", bufs=2, space="PSUM") as ps_proj,
            tc.tile_pool(name="ps_st", bufs=2, space="PSUM") as ps_st,
            tc.tile_pool(name="ps_ctx", bufs=2, space="PSUM") as ps_ctx,
            tc.tile_pool(name="ps_d", bufs=1, space="PSUM") as ps_d,
            tc.tile_pool(name="ps_out", bufs=1, space="PSUM") as ps_out,
        ):
            # constants
            tri32 = constp.tile([128, 128], F32, name="tri32")
            make_upper_triangular(nc, tri32[:], val=1.0, diag=True)
            ones32 = constp.tile([128, 1], F32, name="ones32")
            nc.vector.memset(ones32[:], 1.0)
            ones_r = constp.tile([128, 1], F32R, name="ones_r")
            nc.vector.tensor_copy(ones_r[:], ones32[:])

            for g in range(NG):
                # per-group weight slices [128, NK, 256], d on partitions
                wq_t = wpool.tile([128, NK, G * HD], F32R, tag="wq", name=f"wq{g}")
                wk_t = wpool.tile([128, NK, G * HD], F32R, tag="wk", name=f"wk{g}")
                wv_t = wpool.tile([128, NK, G * HD], F32R, tag="wv", name=f"wv{g}")
                for w_sb, w_dr in ((wq_t, wq), (wk_t, wk), (wv_t, wv)):
                    src = w_dr.ap()[:, g * G * HD:(g + 1) * G * HD]
                    nc.sync.dma_start(
                        w_sb[:], src.rearrange("(o p) n -> p o n", p=128)
                    )

                kt = [
                    ktvp.tile([128, S], F32R, tag=f"kt{t}", name=f"kt{g}_{t}")
                    for t in range(G)
                ]
                v2 = ktvp.tile([128, NK, G * HD], F32R, tag="v2", name=f"v2{g}")
                ctx_g = ctxp.tile([128, G, S], F32R, tag="ctx", name=f"ctx{g}")

                for j in range(NSQ):
                    xts = []
                    for k in range(NK):
                        t_ = xtp.tile([128, SQ], F32R, tag="xt", name=f"x{g}{j}{k}")
                        nc.sync.dma_start(
                            t_[:],
                            xt.ap()[k * 128:(k + 1) * 128, j * SQ:(j + 1) * SQ],
                        )
                        xts.append(t_)

                    # ---- pass Q: QT[t] [hd=128, sq 512] (one PSUM bank at a time)
                    qt = []
                    for t in range(G):
                        pq = ps_proj.tile([128, SQ], F32, tag="proj", name=f"pq{t}")
                        for k in range(NK):
                            nc.tensor.matmul(
                                pq[:],
                                wq_t[:, k, t * HD:(t + 1) * HD],
                                xts[k][:],
                                start=(k == 0),
                                stop=(k == NK - 1),
                            )
                        q_ = qtp.tile([128, SQ], F32R, tag="qt", name=f"qt{t}")
                        nc.vector.tensor_copy(q_[:], pq[:])
                        qt.append(q_)
                    if DEBUG_DUMPS and g == 0 and j == 0:
                        nc.sync.dma_start(dbg["qt"].ap(), qt[0][:])

                    # ---- pass K: KT[t][:, j*SQ:+SQ]
                    for t in range(G):
                        pk = ps_proj.tile([128, SQ], F32, tag="proj", name=f"pk{t}")
                        for k in range(NK):
                            nc.tensor.matmul(
                                pk[:],
                                wk_t[:, k, t * HD:(t + 1) * HD],
                                xts[k][:],
                                start=(k == 0),
                                stop=(k == NK - 1),
                            )
                        nc.vector.tensor_copy(kt[t][:, j * SQ:(j + 1) * SQ], pk[:])

                    # ---- pass V: V[sq 128, 2*HD] for 4 sq-subchunks.
                    # One accumulation group per PSUM bank: start=True clears
                    # the whole bank, so groups must not share one.
                    for s_ in range(4):
                        pv = ps_proj.tile([128, 256], F32, tag="proj", name=f"pv{s_}")
                        for k in range(NK):
                            nc.tensor.matmul(
                                pv[:],
                                xts[k][:, s_ * 128:(s_ + 1) * 128],
                                wv_t[:, k, :],
                                start=(k == 0),
                                stop=(k == NK - 1),
                            )
                        nc.vector.tensor_copy(v2[:, 4 * j + s_, :], pv[:])

                    # ---- attention for both heads at this j
                    n_sk = 4 * (j + 1)
                    for t in range(G):
                        dps = ps_d.tile([1, SQ], F32, tag="d", name="dps")
                        cps = ps_ctx.tile([128, SQ], F32, tag="ctx", name="cps")
                        for i in range(n_sk):
                            r = i - 4 * j  # >=0: straddles the causal diagonal
                            lo = 128 * r if r > 0 else 0
                            st = ps_st.tile([128, SQ], F32, tag="st", name="st")
                            nc.tensor.matmul(
                                st[:, lo:],
                                kt[t][:, i * 128:(i + 1) * 128],
                                qt[t][:, lo:],
                                start=True,
                                stop=True,
                            )
                            pt = ptp.tile([128, SQ], F32R, tag="pt", name="pt")
                            nc.scalar.activation(
                                pt[:, lo:], st[:, lo:], EXP, scale=SCALE
                            )
                            if r >= 0:
                                nc.vector.tensor_tensor(
                                    pt[:, lo:lo + 128],
                                    pt[:, lo:lo + 128],
                                    tri32[:],
                                    MULT,
                                )
                            if DEBUG_DUMPS and g == 0 and t == 0 and j == 0 and i == 0:
                                nc.sync.dma_start(dbg["pt"].ap(), pt[:])
                            nc.tensor.matmul(
                                cps[:, lo:],
                                v2[:, i, t * HD:(t + 1) * HD],
                                pt[:, lo:],
                                start=(i == 0),
                                stop=(i == n_sk - 1),
                            )
                            nc.tensor.matmul(
                                dps[0:1, lo:],
                                ones_r[:],
                                pt[:, lo:],
                                start=(i == 0),
                                stop=(i == n_sk - 1),
                            )
                        # normalize: ctx_g[:, t, j*SQ:+SQ] = cps / d
                        rsb = smallp.tile([1, SQ], F32, tag="rsb", name="rsb")
                        nc.vector.reciprocal_approx_fast(rsb[:], dps[:])
                        if DEBUG_DUMPS and g == 0 and t == 0 and j == 0:
                            nc.sync.dma_start(dbg["rsb"].ap(), rsb[:])
                        rrep = smallp.tile([128, SQ], F32, tag="rrep", name="rrep")
                        nc.gpsimd.partition_broadcast(rrep[:], rsb[:])
                        nc.vector.tensor_tensor(
                            ctx_g[:, t, j * SQ:(j + 1) * SQ], cps[:], rrep[:], MULT
                        )

                if DEBUG_DUMPS and g == 0:
                    nc.sync.dma_start(dbg["kt"].ap(), kt[0][:])
                    nc.sync.dma_start(dbg["v2"].ap(), v2[:])
                    nc.sync.dma_start(dbg["ctx"].ap(), ctx_g[:])

                # ---- per-group output projection: out_g = ctx_g @ Wo_g
                for m in range(4):
                    wo_m = wop.tile([128, G, SQ], F32R, tag="wo", name=f"wo{m}")
                    for t in range(G):
                        row0 = g * G * HD + t * HD
                        nc.sync.dma_start(
                            wo_m[:, t, :],
                            wo.ap()[row0:row0 + 128, m * SQ:(m + 1) * SQ],
                        )
                    for s_ in range(S // 128):
                        ops = ps_out.tile([128, SQ], F32, tag="outp", name="ops")
                        for t in range(G):
                            nc.tensor.matmul(
                                ops[:],
                                ctx_g[:, t, s_ * 128:(s_ + 1) * 128],
                                wo_m[:, t, :],
                                start=(t == 0),
                                stop=(t == G - 1),
                            )
                        osb = osbp.tile([128, SQ], F32, tag="osb", name="osb")
                        nc.vector.tensor_copy(osb[:], ops[:])
                        nc.sync.dma_start(
                            outs[g].ap()[s_ * 128:(s_ + 1) * 128, m * SQ:(m + 1) * SQ],
                            osb[:],
                        )

    nc.compile()
    return nc


_NC = None


def _get_nc():
    global _NC
    if _NC is None:
        _NC = _build()
    return _NC


def kernel(x, W_q, W_k, W_v, W_o, b_o):
    x = np.asarray(x, dtype=np.float32)
    W_q = np.asarray(W_q, dtype=np.float32)
    W_k = np.asarray(W_k, dtype=np.float32)
    W_v = np.asarray(W_v, dtype=np.float32)
    W_o = np.asarray(W_o, dtype=np.float32)
    b_o = np.asarray(b_o, dtype=np.float32)

    nc = _get_nc()
    in_maps = []
    for c in range(8):
        b, hg = divmod(c, 2)
        lo = hg * DH
        in_maps.append(
            {
                "xt": np.ascontiguousarray(x[b].T),
                "wq": np.ascontiguousarray(W_q[:, lo:lo + DH]),
                "wk": np.ascontiguousarray(W_k[:, lo:lo + DH]),
                "wv": np.ascontiguousarray(W_v[:, lo:lo + DH]),
                "wo": np.ascontiguousarray(W_o[lo:lo + DH, :]),
            }
        )

    res = run_bass_kernel_spmd(nc, in_maps, core_ids=list(range(8)))

    out = np.zeros((B, S, D), dtype=np.float32)
    for c in range(8):
        b = c // 2
        r = res.results[c]
        for g in range(NG):
            out[b] += r[f"out{g}"]
    out += b_o[None, None, :]
    return out
